# revision 3
# baseline (speedup 1.0000x reference)
"""Trainium2 Bass kernel for nn_Attention_50637664420407 — v2 (bf16).

Dense causal transformer block: LayerNorm -> QKV -> RoPE -> causal attention
-> out-projection.  x:[2,2048,1024] f32.

Sharding (8 cores): head-parallel.  Core c owns heads {2c, 2c+1} for both
batch elements.  W_qkv split column-wise per head group, W_out row-wise; each
core computes a full [4096,1024] bf16 partial of the output projection and
the host sums the 8 partials in f32.

v2 highlights vs v1:
- bf16 data path end-to-end (x, W, q/k/v, attention probs, output partials):
  halves DMA bytes, removes fp32r small-free-dim matmul penalties, 2x DVE.
- LayerNorm mean-correction folded into the weights on the host:
  W' = (W*ln_g) - colmean(W*ln_g), so qkv = rstd * (W'^T x) exactly; only
  rstd (from S1/S2 ones-matmul stats) remains on device.  rstd is folded
  into the RoPE cos/sin tables (ccr/ssr) for q/k and applied to v directly.
- Causal mask applied as a post-exp 0/1 multiply on the diagonal 128-block
  (gpsimd); per-key rstd is folded into the softmax via the exp bias
  ln(rstd_j), with 1/rstd_j stored in the V ones-column so the denominator
  row stays unscaled.
- Softmax denominator: 1/rstd column in V -> 65th AV row; reciprocal on
  DVE, broadcast across 64 partitions via a tiny PE matmul, applied in the
  AV eviction multiply.  No DRAM round-trips.
- QKV (batch 1) and attention (batch 0) interleaved in emission order so PE
  stays busy while ACT runs softmax exps; out-projection of batch b overlaps
  attention of batch b+1.
"""

import sys
import numpy as np

for _p in ("/opt/trn_rl_repo", "/root/.axon_site/_ro/trn_rl_repo"):
    if _p not in sys.path:
        sys.path.append(_p)

import ml_dtypes
import concourse.bass as bass
import concourse.bacc as bacc
import concourse.mybir as mybir
import concourse.tile as tile
from concourse.alu_op_type import AluOpType

F32 = mybir.dt.float32
F32R = mybir.dt.float32r
BF = mybir.dt.bfloat16
AF = mybir.ActivationFunctionType

P = 128          # partitions
T = 4096         # total tokens (2 batches x 2048)
NT = 2048        # seq len per batch
DIMK = 1024      # model dim
KC = 8           # k chunks of 128
TT = 8           # token tiles of 512
D = 64           # head dim
SCALE = D ** -0.5  # 0.125

BF_NP = ml_dtypes.bfloat16


def ts(i, n):
    return slice(i * n, (i + 1) * n)


def r32(ap):
    return ap.bitcast(F32R)


class _Bacc(bacc.Bacc):
    """Bacc with a pinned ACT table-set choice (single table load for the
    whole kernel: natural_log_exp_and_others covers Exp, Ln, Square, Copy)."""

    def insert_act_table_loads(self):
        import concourse.bass_isa as bass_isa  # noqa: F401
        from concourse.hw_specs import get_activation_tables
        import bass_rust as _bass_rust
        has_activation = any(
            isinstance(i, mybir.InstActivation)
            for b in self.main_func.blocks
            for i in b.instructions
        )
        if not has_activation:
            return
        pinned = {AF.Exp, AF.Ln, AF.Square}
        keep = "natural_log_exp_and_others"
        tables = []
        for name, funcs in get_activation_tables(self.m.arch).items():
            if name != keep:
                funcs = funcs - pinned
            tables.append((name, funcs))
        _bass_rust.insert_act_table_loads(self, tables)


def build_program():
    """Single-core SPMD Bass program (same program on all 8 cores)."""
    nc = _Bacc("TRN2", target_bir_lowering=False, debug=False)

    xt_h = nc.declare_dram_parameter("xt", [DIMK, T], BF, False)
    wq_h = nc.declare_dram_parameter("wqkv", [DIMK, 384], BF, False)
    wo_h = nc.declare_dram_parameter("wo", [P, DIMK], BF, False)
    cc_h = nc.declare_dram_parameter("cc", [P, NT], BF, False)   # cos, 2-head tiled
    ss_h = nc.declare_dram_parameter("ss", [P, NT], BF, False)   # signed sin
    trin_h = nc.declare_dram_parameter("trin", [P, P], BF, False)  # causal 0/1
    perm_h = nc.declare_dram_parameter("perm", [P, P], BF, False)  # rotate-half
    out_h = nc.declare_dram_parameter("out", [T, DIMK], BF, True)

    with tile.TileContext(nc) as tc:
        with tc.tile_pool(name="const", bufs=1) as const, \
             tc.tile_pool(name="qkvsb", bufs=1) as qkvsb, \
             tc.tile_pool(name="ohp", bufs=1) as ohp, \
             tc.tile_pool(name="vbp", bufs=1) as vbp, \
             tc.tile_pool(name="pp", bufs=1) as pp, \
             tc.tile_pool(name="stg", bufs=2) as stg, \
             tc.tile_pool(name="rrec", bufs=2) as rrec, \
             tc.tile_pool(name="scp", bufs=2, space="PSUM") as scp, \
             tc.tile_pool(name="avp", bufs=1, space="PSUM") as avp:

            # ---------- persistent sbuf tensors (DMAs emitted later, after x) --
            w_t = const.tile([P, KC, 384], BF)
            wo_t = const.tile([P, DIMK], BF)
            cc_t = const.tile([P, NT], BF)
            ss_t = const.tile([P, NT], BF)
            trin_t = const.tile([P, P], BF)
            perm_t = const.tile([P, P], BF)
            ones128 = const.tile([P, 1], BF)
            nc.gpsimd.memset(ones128, 1.0)
            ones1b = const.tile([1, P], BF)
            nc.gpsimd.memset(ones1b, 1.0)
            eps1 = const.tile([1, 1], F32)
            nc.vector.memset(eps1, 1e-5)

            # qn/kn/vn: feature-major [2 heads x 64, tokens]; q/k rotated+scaled
            qn = qkvsb.tile([P, T], BF)
            kn = qkvsb.tile([P, T], BF)
            vn = qkvsb.tile([P, T], BF)
            ohT = ohp.tile([P, T], BF)        # attention out (feature-major)
            # vb: token-major V (unscaled) per (head, global 128-tok block);
            # 65th column holds 1/rstd_j so the denominator row comes out
            # unscaled while pJ carries rstd_j via the exp bias ln(rstd_j)
            vb = vbp.tile([P, 2, 32, D + 1], BF)
            # token-transposed stats per 128-token block: col 0 = ln(rstd),
            # col 1 = 1/rstd
            lnrv = vbp.tile([P, 32, 2], BF)
            # lnriv staging buffers (rows 0/32 live, rest junk) — memset once
            # so the transpose never reads uninitialized SBUF
            lnriv_bufs = [const.tile([D, 512], BF, name=f"lnriv{i}")
                          for i in range(2)]
            for _b in lnriv_bufs:
                nc.gpsimd.memset(_b, 0.0)

            # x chunk tiles, [128, 1024] per (chunk, 2-tile group)
            xg = {}

            def xdma(g):
                for k in range(KC):
                    t_ = xtc.tile([P, 1024], BF, tag=f"x{k}")
                    nc.sync.dma_start(out=t_, in_=xt_h[ts(k, P), ts(g, 1024)])
                    xg[(k, g)] = t_

            stats_out = {}

            def stats_stage(t):
                """Squares + S1/S2 + stats math + ACT chain for tile t.
                Runs one tile ahead of main_stage so the rstd chain never
                blocks the main tile's rope."""
                g, half = t // 2, t % 2
                xts = [xg[(k, g)][:, ts(half, 512)] for k in range(KC)]
                sqs = []
                if half == 0:
                    for k in range(4):
                        sq = sqp.tile([P, 1024], BF, tag=f"sq{k}")
                        nc.gpsimd.tensor_mul(sq, xg[(k, g)], xg[(k, g)])
                        sqg[k] = sq
                for k in range(KC):
                    if k < 4:
                        sqs.append(sqg[k][:, ts(half, 512)])
                    else:
                        sq = sqp.tile([P, 512], BF, tag=f"sq{k}")
                        nc.vector.tensor_mul(sq, xts[k], xts[k])
                        sqs.append(sq)
                s12 = avp.tile([65, 512], F32, tag="av", name=f"s12_{t}")
                for k in range(KC):
                    nc.tensor.matmul(s12[0:1, :], lhsT=ones128, rhs=xts[k],
                                     start=(k == 0), stop=(k == KC - 1))
                for k in range(KC):
                    nc.tensor.matmul(s12[64:65, :], lhsT=ones128, rhs=sqs[k],
                                     start=(k == 0), stop=(k == KC - 1))
                # stats: rstd = exp(-0.5*ln(var+eps)); var = S2/n - (S1/n)^2
                m1 = stg.tile([1, 512], BF, tag="m1")
                if t < 4:
                    nc.scalar.activation(out=m1, in_=s12[0:1, :], func=AF.Copy,
                                         scale=1.0 / DIMK)
                else:
                    nc.vector.tensor_scalar_mul(m1, in0=s12[0:1, :],
                                                scalar1=1.0 / DIMK)
                t2 = stg.tile([1, 512], BF, tag="t2")
                nc.vector.tensor_mul(t2, m1, m1)
                lv = stg.tile([1, 512], F32, tag="lv")
                nc.vector.scalar_tensor_tensor(out=lv, in0=s12[64:65, :],
                                               scalar=1.0 / DIMK, in1=t2,
                                               op0=AluOpType.mult,
                                               op1=AluOpType.subtract)
                nc.scalar.activation(out=lv, in_=lv, func=AF.Ln, bias=eps1)
                rs = stg.tile([1, 512], BF, tag="rs")
                nc.scalar.activation(out=rs, in_=lv, func=AF.Exp, scale=-0.5)
                # lnriv rows 0/32 = ln(rstd), 1/rstd (engine writes must land
                # on partition 0/32/64); other rows carry stale-but-initialized
                # data that lands in unread transpose columns
                lnriv = lnriv_bufs[t % 2]
                nc.scalar.activation(out=lnriv[32:33, :], in_=lv, func=AF.Exp,
                                     scale=0.5)
                nc.vector.tensor_scalar_mul(lnriv[0:1, :], in0=lv, scalar1=-0.5)
                stats_out[t] = (rs, lnriv)

            def main_stage(t):
                """QKV + RoPE + V transpose for tile t (stats precomputed)."""
                g, half = t // 2, t % 2
                xts = [xg[(k, g)][:, ts(half, 512)] for k in range(KC)]
                rs, lnriv = stats_out.pop(t)
                # broadcast rstd across partitions via a tiny PE outer-product
                rb = qkvp.tile([P, 512], F32, tag="qkv", name=f"rb_{t}")
                nc.tensor.matmul(rb, lhsT=ones1b, rhs=rs,
                                 start=True, stop=True)
                # rstd-scaled rope tables (DVE, early in this unit's queue)
                cs = ts(t % 4, 512)
                ccr = stg.tile([P, 512], BF, tag="ccr")
                nc.vector.tensor_mul(ccr, cc_t[:, cs], rb)
                ssr = stg.tile([P, 512], BF, tag="ssr")
                nc.vector.tensor_mul(ssr, ss_t[:, cs], rb)
                qps = qkvp.tile([P, 512], F32, tag="qkv", name="qps")
                for k in range(KC):
                    nc.tensor.matmul(qps, lhsT=w_t[:, k, ts(0, P)], rhs=xts[k],
                                     start=(k == 0), stop=(k == KC - 1))
                if t < 4:
                    nc.scalar.copy(qn[:, ts(t, 512)], qps)
                else:
                    nc.vector.tensor_copy(qn[:, ts(t, 512)], qps)
                kps = qkvp.tile([P, 512], F32, tag="qkv", name="kps")
                for k in range(KC):
                    nc.tensor.matmul(kps, lhsT=w_t[:, k, ts(1, P)], rhs=xts[k],
                                     start=(k == 0), stop=(k == KC - 1))
                if t < 4:
                    nc.scalar.copy(kn[:, ts(t, 512)], kps)
                else:
                    nc.vector.tensor_copy(kn[:, ts(t, 512)], kps)
                vps = qkvp.tile([P, 512], F32, tag="qkv", name="vps")
                for k in range(KC):
                    nc.tensor.matmul(vps, lhsT=w_t[:, k, ts(2, P)], rhs=xts[k],
                                     start=(k == 0), stop=(k == KC - 1))
                if t < 4:
                    nc.scalar.copy(vn[:, ts(t, 512)], vps)
                else:
                    nc.vector.tensor_copy(vn[:, ts(t, 512)], vps)

                def rope(src):
                    sl = src[:, ts(t, 512)]
                    rp = qkvp.tile([P, 512], F32, tag="qkv")
                    nc.tensor.matmul(rp, lhsT=perm_t, rhs=sl, start=True, stop=True)
                    ra = stg.tile([P, 512], BF, tag="ra")
                    nc.vector.tensor_mul(ra, sl, ccr)
                    rb2 = stg.tile([P, 512], BF, tag="rb2")
                    nc.vector.tensor_mul(rb2, rp, ssr)
                    nc.vector.tensor_add(sl, ra, rb2)

                rope(qn)
                # V "transpose" via regular matmul against identity: output
                # lands in PSUM as f32, which DVE can then 3D-copy to bf16
                # (strided bf16-PSUM reads are rejected by the backend).
                # All matmul groups within one PSUM bank must share the same
                # operand base partition, so head-0 (+lnriv) and head-1
                # blocks go to separate banks.
                tpA = qkvp.tile([P, 8, D], F32, tag="qkv", name="tpA")
                tpB = qkvp.tile([P, 4, D], F32, tag="qkv", name="tpB")
                for j in range(4):
                    nc.tensor.matmul(
                        tpA[:, j, :],
                        lhsT=vn[0:D, 512 * t + P * j : 512 * t + P * (j + 1)],
                        rhs=ident[0:D, 0:D],
                        start=True, stop=True)
                for j in range(4):
                    nc.tensor.matmul(
                        tpA[:, 4 + j, :],
                        lhsT=lnriv[:, P * j : P * (j + 1)],
                        rhs=ident[0:D, 0:D],
                        start=True, stop=True)
                for j in range(4):
                    nc.tensor.matmul(
                        tpB[:, j, :],
                        lhsT=vn[D : 2 * D,
                                512 * t + P * j : 512 * t + P * (j + 1)],
                        rhs=ident[D : 2 * D, D : 2 * D],
                        start=True, stop=True)
                nc.vector.tensor_copy(vb[:, 0, 4 * t : 4 * t + 4, 0:D],
                                      tpA[:, 0:4, :])
                nc.vector.tensor_copy(lnrv[:, 4 * t : 4 * t + 4, 0:1],
                                      tpA[:, 4:8, 0:1])
                nc.vector.tensor_copy(lnrv[:, 4 * t : 4 * t + 4, 1:2],
                                      tpA[:, 4:8, 32:33])
                nc.vector.tensor_copy(vb[:, 1, 4 * t : 4 * t + 4, 0:D],
                                      tpB[:, 0:4, :])
                for h in range(2):
                    nc.gpsimd.tensor_copy(vb[:, h, 4 * t : 4 * t + 4, D : D + 1],
                                          lnrv[:, 4 * t : 4 * t + 4, 1:2])
                rope(kn)

            def sc_group(b, h, J):
                """Scores + exp for key block J of pair (b,h): fills pJ."""
                base = NT * b
                hr = D * h
                m = J % 4
                i0 = 512 * (J // 4)
                ilen = NT - i0
                off0 = P * m
                pJ = pp.tile([P, ilen], BF, tag=f"p{h}_{J}")
                pjs[(h, J)] = pJ
                lhs = kn[hr : hr + D, base + P * J : base + P * (J + 1)]
                for c0 in range(0, ilen, 1024):
                    clen = min(1024, ilen - c0)
                    win = scp.tile([P, 1024], F32, tag="sc")
                    first = c0 == 0
                    lo = off0 if first else 0
                    b0 = lo
                    while b0 < clen:
                        n = min(512 - (b0 % 512), clen - b0)
                        nc.tensor.matmul(
                            win[:, b0 : b0 + n], lhsT=lhs,
                            rhs=qn[hr : hr + D,
                                   base + i0 + c0 + b0 : base + i0 + c0 + b0 + n],
                            start=True, stop=True)
                        b0 += n
                    nc.scalar.activation(out=pJ[:, c0 + lo : c0 + clen],
                                         in_=win[:, lo:clen],
                                         func=AF.Exp, scale=SCALE,
                                         bias=lnrv[:, 16 * b + J, 0:1])
                # causal mask on the diagonal 128x128 block (post-exp)
                nc.gpsimd.tensor_mul(pJ[:, off0 : off0 + P],
                                     pJ[:, off0 : off0 + P], trin_t)

            def av_group(b, h, I):
                """AV for query block I of pair (b,h) + denominator handling."""
                base = NT * b
                hr = D * h
                av = avp.tile([D + 1, 512], F32, tag="av")
                last = 4 * I + 3
                for J in range(0, last + 1):
                    i0 = 512 * (J // 4)
                    cbase = 512 * I - i0
                    off = P * (J % 4) if J // 4 == I else 0
                    nc.tensor.matmul(
                        av[:, off:512],
                        lhsT=vb[:, h, 16 * b + J, :],
                        rhs=pjs[(h, J)][:, cbase + off : cbase + 512],
                        start=(J == 0), stop=(J == last))
                rec = rrec.tile([1, 512], BF, tag="rec")
                with nc.allow_low_precision("softmax denom reciprocal in bf16"):
                    nc.vector.reciprocal(rec, av[D : D + 1, :])
                recb = avp.tile([D, 512], F32, tag="recb")
                nc.tensor.matmul(recb, lhsT=ones1b[:, 0:D], rhs=rec,
                                 start=True, stop=True)
                recb_b = rrec.tile([D, 512], BF, tag="recb_b")
                nc.vector.tensor_copy(recb_b, recb)
                nc.vector.tensor_mul(
                    ohT[hr : hr + D, base + 512 * I : base + 512 * (I + 1)],
                    av[0:D, :], recb_b)

            def op_tile(tt_, eng, pool=None):
                """Out-projection for 128-token tile tt_ (0..31)."""
                ps = (pool or opp).tile([P, DIMK], F32, tag="op" if pool is None else "sc")
                for cb in range(2):
                    nc.tensor.matmul(ps[:, ts(cb, 512)],
                                     lhsT=ohT[:, ts(tt_, P)],
                                     rhs=wo_t[:, ts(cb, 512)],
                                     start=True, stop=True)
                ev = evp.tile([P, DIMK], BF, tag="ev")
                if eng == 0:
                    nc.vector.tensor_copy(ev, ps)
                else:
                    nc.vector.tensor_copy(ev[:, 0:512], ps[:, 0:512])
                    nc.scalar.copy(ev[:, 512:1024], ps[:, 512:1024])
                nc.sync.dma_start(out=out_h[ts(tt_, P), :], in_=ev)

            pjs = {}
            sqg = {}

            with tc.tile_pool(name="xtc", bufs=2) as xtc, \
                 tc.tile_pool(name="sqp", bufs=1) as sqp, \
                 tc.tile_pool(name="qkvp", bufs=2, space="PSUM") as qkvp:

                ident = const.tile([P, P], BF)
                nc.gpsimd.memset(ident, 0.0)
                nc.gpsimd.affine_select(out=ident, in_=ident,
                                        compare_op=AluOpType.not_equal, fill=1.0,
                                        base=0, pattern=[[-1, P]],
                                        channel_multiplier=1)

                # startup: w chunk k then x chunk k, so PE starts at chunk 0
                for k in range(KC):
                    nc.sync.dma_start(out=w_t[:, k, :], in_=wq_h[ts(k, P), :])
                    t_ = xtc.tile([P, 1024], BF, tag=f"x{k}")
                    nc.sync.dma_start(out=t_, in_=xt_h[ts(k, P), ts(0, 1024)])
                    xg[(k, 0)] = t_
                nc.sync.dma_start(out=cc_t, in_=cc_h[:, :])
                nc.sync.dma_start(out=ss_t, in_=ss_h[:, :])
                nc.sync.dma_start(out=perm_t, in_=perm_h[:, :])
                xdma(1)
                nc.sync.dma_start(out=trin_t, in_=trin_h[:, :])
                nc.sync.dma_start(out=wo_t, in_=wo_h[:, :])
                stats_stage(0)
                stats_stage(1)
                main_stage(0)
                stats_stage(2)
                xdma(2)
                main_stage(1)
                stats_stage(3)
                main_stage(2)
                stats_stage(4)
                xdma(3)
                main_stage(3)
                stats_stage(5)

                # batch-0 attention interleaved with batch-1 QKV units
                sc_group(0, 0, 0)
                sc_group(0, 1, 0)
                sc_group(0, 0, 1)
                sc_group(0, 1, 1)
                main_stage(4)
                stats_stage(6)
                sc_group(0, 0, 2)
                sc_group(0, 1, 2)
                sc_group(0, 0, 3)
                sc_group(0, 1, 3)
                main_stage(5)
                stats_stage(7)
                sc_group(0, 0, 4)
                sc_group(0, 1, 4)
                sc_group(0, 0, 5)
                sc_group(0, 1, 5)
                main_stage(6)
                sc_group(0, 0, 6)
                sc_group(0, 1, 6)
                sc_group(0, 0, 7)
                sc_group(0, 1, 7)
                main_stage(7)
                av_group(0, 0, 0)
                sc_group(0, 0, 8)
                av_group(0, 1, 0)
                sc_group(0, 1, 8)
                av_group(0, 0, 1)
                sc_group(0, 0, 9)
                av_group(0, 1, 1)
                sc_group(0, 1, 9)
                for J in range(10, 12):
                    sc_group(0, 0, J)
                    sc_group(0, 1, J)
                av_group(0, 0, 2)
                sc_group(0, 0, 12)
                av_group(0, 1, 2)
                sc_group(0, 1, 12)
                for J in range(13, 16):
                    sc_group(0, 0, J)
                    sc_group(0, 1, J)
                av_group(0, 0, 3)
                av_group(0, 1, 3)

            # batch-1 attention with batch-0 out-projection as filler
            with tc.tile_pool(name="opp", bufs=1, space="PSUM") as opp, \
                 tc.tile_pool(name="evp", bufs=2) as evp:
                # out-proj tiles spread one-per-slot between sc groups so the
                # single opp PSUM buffer never stalls PE on its evict
                for J in range(8):
                    sc_group(1, 0, J)
                    op_tile(J, J % 2)
                    sc_group(1, 1, J)
                    op_tile(8 + J, (J + 1) % 2)
                av_group(1, 0, 0)
                sc_group(1, 0, 8)
                av_group(1, 1, 0)
                sc_group(1, 1, 8)
                op_tile(16, 0)
                av_group(1, 0, 1)
                sc_group(1, 0, 9)
                op_tile(17, 1)
                av_group(1, 1, 1)
                sc_group(1, 1, 9)
                op_tile(18, 0)
                sc_group(1, 0, 10)
                op_tile(19, 1)
                sc_group(1, 1, 10)
                sc_group(1, 0, 11)
                sc_group(1, 1, 11)
                av_group(1, 0, 2)
                sc_group(1, 0, 12)
                op_tile(20, 0)
                av_group(1, 1, 2)
                sc_group(1, 1, 12)
                op_tile(21, 1)
                sc_group(1, 0, 13)
                op_tile(22, 0)
                sc_group(1, 1, 13)
                op_tile(23, 1)
                sc_group(1, 0, 14)
                op_tile(24, 0)
                sc_group(1, 1, 14)
                op_tile(25, 1)
                sc_group(1, 0, 15)
                op_tile(26, 0)
                av_group(1, 0, 3)
                sc_group(1, 1, 15)
                op_tile(27, 1)
                av_group(1, 1, 3)
                op_tile(28, 0)
                op_tile(29, 1)
                op_tile(30, 0)
                op_tile(31, 1)

    nc.finalize()
    return nc


def host_inputs(x, W_qkv, W_out, ln_g, ln_b):
    """Per-core input maps (layout/sharding + LN weight folding)."""
    x = np.asarray(x, dtype=np.float32)
    W_qkv = np.asarray(W_qkv, dtype=np.float32)
    W_out = np.asarray(W_out, dtype=np.float32)
    ln_g = np.asarray(ln_g, dtype=np.float32)
    ln_b = np.asarray(ln_b, dtype=np.float32)
    assert np.all(ln_b == 0.0), "bias path not emitted"

    xt = np.ascontiguousarray(x.reshape(T, DIMK).T).astype(BF_NP)  # [1024,4096]

    inv_freq = (1.0 / (10000.0 ** (np.arange(0, D, 2, dtype=np.float32) / D))).astype(np.float32)
    tpos = np.arange(NT, dtype=np.float32)
    freqs = np.outer(tpos, inv_freq).astype(np.float32)     # [2048, 32]
    emb = np.concatenate([freqs, freqs], axis=1)            # [2048, 64]
    cosT = np.cos(emb).T.astype(np.float32)                 # [64, 2048]
    sinT = np.sin(emb).T.astype(np.float32)
    ss_signed = np.concatenate([-sinT[:32], sinT[32:]], axis=0)
    cc = np.ascontiguousarray(np.tile(cosT, (2, 1))).astype(BF_NP)
    ss = np.ascontiguousarray(np.tile(ss_signed, (2, 1))).astype(BF_NP)
    trin = (np.arange(P)[None, :] >= np.arange(P)[:, None]).astype(BF_NP)
    perm = np.zeros((P, P), np.float32)
    for m in range(P):
        blk = (m // D) * D
        perm[blk + (m % D + 32) % D, m] = 1.0
    perm = perm.astype(BF_NP)

    Wg = W_qkv * ln_g[:, None]

    in_maps = []
    for c in range(8):
        cols = np.concatenate([np.arange(P * c, P * (c + 1)),
                               1024 + np.arange(P * c, P * (c + 1)),
                               2048 + np.arange(P * c, P * (c + 1))])
        wl = Wg[:, cols]
        wl = wl - wl.mean(axis=0, keepdims=True)     # fold LN mean-correction
        in_maps.append({
            "xt": xt,
            "wqkv": np.ascontiguousarray(wl).astype(BF_NP),
            "wo": np.ascontiguousarray(W_out[P * c : P * (c + 1), :]).astype(BF_NP),
            "cc": cc, "ss": ss, "trin": trin, "perm": perm,
        })
    return in_maps


_NC_CACHE = {}


def get_program():
    if "nc" not in _NC_CACHE:
        _NC_CACHE["nc"] = build_program()
    return _NC_CACHE["nc"]


LAST_RESULTS = {}


def kernel(x, W_qkv, W_out, b_out, ln_g, ln_b):
    import os
    from concourse.bass_utils import run_bass_kernel_spmd
    nc = get_program()
    in_maps = host_inputs(x, W_qkv, W_out, ln_g, ln_b)
    kw = {}
    if os.environ.get("BASS_KERNEL_TMPDIR"):
        kw["tmpdir"] = os.environ["BASS_KERNEL_TMPDIR"]
    res = run_bass_kernel_spmd(nc, in_maps, list(range(8)), **kw)
    LAST_RESULTS["res"] = res
    total = np.zeros((T, DIMK), dtype=np.float32)
    for r in res.results:
        total += np.asarray(r["out"], dtype=np.float32)
    total += np.asarray(b_out, dtype=np.float32)[None, :]
    return total.reshape(2, NT, DIMK)


# revision 4
# speedup vs baseline: 1.0078x; 1.0078x over previous
"""Trainium2 Bass kernel for nn_Attention_50637664420407 — v2 (bf16).

Dense causal transformer block: LayerNorm -> QKV -> RoPE -> causal attention
-> out-projection.  x:[2,2048,1024] f32.

Sharding (8 cores): head-parallel.  Core c owns heads {2c, 2c+1} for both
batch elements.  W_qkv split column-wise per head group, W_out row-wise; each
core computes a full [4096,1024] bf16 partial of the output projection and
the host sums the 8 partials in f32.

v2 highlights vs v1:
- bf16 data path end-to-end (x, W, q/k/v, attention probs, output partials):
  halves DMA bytes, removes fp32r small-free-dim matmul penalties, 2x DVE.
- LayerNorm mean-correction folded into the weights on the host:
  W' = (W*ln_g) - colmean(W*ln_g), so qkv = rstd * (W'^T x) exactly; only
  rstd (from S1/S2 ones-matmul stats) remains on device.  rstd is folded
  into the RoPE cos/sin tables (ccr/ssr) for q/k and applied to v directly.
- Causal mask applied as a post-exp 0/1 multiply on the diagonal 128-block
  (gpsimd); per-key rstd is folded into the softmax via the exp bias
  ln(rstd_j), with 1/rstd_j stored in the V ones-column so the denominator
  row stays unscaled.
- Softmax denominator: 1/rstd column in V -> 65th AV row; reciprocal on
  DVE, broadcast across 64 partitions via a tiny PE matmul, applied in the
  AV eviction multiply.  No DRAM round-trips.
- QKV (batch 1) and attention (batch 0) interleaved in emission order so PE
  stays busy while ACT runs softmax exps; out-projection of batch b overlaps
  attention of batch b+1.
"""

import sys
import numpy as np

for _p in ("/opt/trn_rl_repo", "/root/.axon_site/_ro/trn_rl_repo"):
    if _p not in sys.path:
        sys.path.append(_p)

import ml_dtypes
import concourse.bass as bass
import concourse.bacc as bacc
import concourse.mybir as mybir
import concourse.tile as tile
from concourse.alu_op_type import AluOpType

F32 = mybir.dt.float32
F32R = mybir.dt.float32r
BF = mybir.dt.bfloat16
AF = mybir.ActivationFunctionType

P = 128          # partitions
T = 4096         # total tokens (2 batches x 2048)
NT = 2048        # seq len per batch
DIMK = 1024      # model dim
KC = 8           # k chunks of 128
TT = 8           # token tiles of 512
D = 64           # head dim
SCALE = D ** -0.5  # 0.125

BF_NP = ml_dtypes.bfloat16


def ts(i, n):
    return slice(i * n, (i + 1) * n)


def r32(ap):
    return ap.bitcast(F32R)


class _Bacc(bacc.Bacc):
    """Bacc with a pinned ACT table-set choice (single table load for the
    whole kernel: natural_log_exp_and_others covers Exp, Ln, Square, Copy)."""

    def insert_act_table_loads(self):
        import concourse.bass_isa as bass_isa  # noqa: F401
        from concourse.hw_specs import get_activation_tables
        import bass_rust as _bass_rust
        has_activation = any(
            isinstance(i, mybir.InstActivation)
            for b in self.main_func.blocks
            for i in b.instructions
        )
        if not has_activation:
            return
        pinned = {AF.Exp, AF.Ln, AF.Square}
        keep = "natural_log_exp_and_others"
        tables = []
        for name, funcs in get_activation_tables(self.m.arch).items():
            if name != keep:
                funcs = funcs - pinned
            tables.append((name, funcs))
        _bass_rust.insert_act_table_loads(self, tables)


def build_program():
    """Single-core SPMD Bass program (same program on all 8 cores)."""
    nc = _Bacc("TRN2", target_bir_lowering=False, debug=False)

    xt_h = nc.declare_dram_parameter("xt", [DIMK, T], BF, False)
    wq_h = nc.declare_dram_parameter("wqkv", [DIMK, 384], BF, False)
    wo_h = nc.declare_dram_parameter("wo", [P, DIMK], BF, False)
    cc_h = nc.declare_dram_parameter("cc", [P, NT], BF, False)   # cos, 2-head tiled
    ss_h = nc.declare_dram_parameter("ss", [P, NT], BF, False)   # signed sin
    trin_h = nc.declare_dram_parameter("trin", [P, P], BF, False)  # causal 0/1
    perm_h = nc.declare_dram_parameter("perm", [P, P], BF, False)  # rotate-half
    out_h = nc.declare_dram_parameter("out", [T, DIMK], BF, True)

    with tile.TileContext(nc) as tc:
        with tc.tile_pool(name="const", bufs=1) as const, \
             tc.tile_pool(name="qkvsb", bufs=1) as qkvsb, \
             tc.tile_pool(name="ohp", bufs=1) as ohp, \
             tc.tile_pool(name="vbp", bufs=1) as vbp, \
             tc.tile_pool(name="pp", bufs=1) as pp, \
             tc.tile_pool(name="stg", bufs=2) as stg, \
             tc.tile_pool(name="rrec", bufs=2) as rrec, \
             tc.tile_pool(name="scp", bufs=2, space="PSUM") as scp, \
             tc.tile_pool(name="avp", bufs=1, space="PSUM") as avp:

            # ---------- persistent sbuf tensors (DMAs emitted later, after x) --
            w_t = const.tile([P, KC, 384], BF)
            wo_t = const.tile([P, DIMK], BF)
            cc_t = const.tile([P, NT], BF)
            ss_t = const.tile([P, NT], BF)
            trin_t = const.tile([P, P], BF)
            perm_t = const.tile([P, P], BF)
            ones128 = const.tile([P, 1], BF)
            nc.gpsimd.memset(ones128, 1.0)
            ones1b = const.tile([1, P], BF)
            nc.gpsimd.memset(ones1b, 1.0)
            eps1 = const.tile([1, 1], F32)
            nc.vector.memset(eps1, 1e-5)

            # qn/kn/vn: feature-major [2 heads x 64, tokens]; q/k rotated+scaled
            qn = qkvsb.tile([P, T], BF)
            kn = qkvsb.tile([P, T], BF)
            vn = qkvsb.tile([P, T], BF)
            ohT = ohp.tile([P, T], BF)        # attention out (feature-major)
            # vb: token-major V (unscaled) per (head, global 128-tok block);
            # 65th column holds 1/rstd_j so the denominator row comes out
            # unscaled while pJ carries rstd_j via the exp bias ln(rstd_j)
            vb = vbp.tile([P, 2, 32, D + 1], BF)
            # token-transposed stats per 128-token block: col 0 = ln(rstd),
            # col 1 = 1/rstd
            lnrv = vbp.tile([P, 32, 2], BF)
            # lnriv staging buffers (rows 0/32 live, rest junk) — memset once
            # so the transpose never reads uninitialized SBUF
            lnriv_bufs = [const.tile([D, 512], BF, name=f"lnriv{i}")
                          for i in range(2)]
            for _b in lnriv_bufs:
                nc.gpsimd.memset(_b, 0.0)

            # x chunk tiles, [128, 1024] per (chunk, 2-tile group)
            xg = {}

            def xdma(g):
                for k in range(KC):
                    t_ = xtc.tile([P, 1024], BF, tag=f"x{k}")
                    nc.sync.dma_start(out=t_, in_=xt_h[ts(k, P), ts(g, 1024)])
                    xg[(k, g)] = t_

            stats_out = {}

            def stats_stage(t):
                """Squares + S1/S2 + stats math + ACT chain for tile t.
                Runs one tile ahead of main_stage so the rstd chain never
                blocks the main tile's rope."""
                g, half = t // 2, t % 2
                xts = [xg[(k, g)][:, ts(half, 512)] for k in range(KC)]
                sqs = []
                if half == 0:
                    for k in range(4):
                        sq = sqp.tile([P, 1024], BF, tag=f"sq{k}")
                        nc.gpsimd.tensor_mul(sq, xg[(k, g)], xg[(k, g)])
                        sqg[k] = sq
                for k in range(KC):
                    if k < 4:
                        sqs.append(sqg[k][:, ts(half, 512)])
                    else:
                        sq = sqp.tile([P, 512], BF, tag=f"sq{k}")
                        nc.vector.tensor_mul(sq, xts[k], xts[k])
                        sqs.append(sq)
                s12 = avp.tile([65, 512], F32, tag="av", name=f"s12_{t}")
                for k in range(KC):
                    nc.tensor.matmul(s12[0:1, :], lhsT=ones128, rhs=xts[k],
                                     start=(k == 0), stop=(k == KC - 1))
                for k in range(KC):
                    nc.tensor.matmul(s12[64:65, :], lhsT=ones128, rhs=sqs[k],
                                     start=(k == 0), stop=(k == KC - 1))
                # stats: rstd = exp(-0.5*ln(var+eps)); var = S2/n - (S1/n)^2
                m1 = stg.tile([1, 512], BF, tag="m1")
                if t < 4:
                    nc.scalar.activation(out=m1, in_=s12[0:1, :], func=AF.Copy,
                                         scale=1.0 / DIMK)
                else:
                    nc.vector.tensor_scalar_mul(m1, in0=s12[0:1, :],
                                                scalar1=1.0 / DIMK)
                t2 = stg.tile([1, 512], BF, tag="t2")
                nc.vector.tensor_mul(t2, m1, m1)
                lv = stg.tile([1, 512], F32, tag="lv")
                nc.vector.scalar_tensor_tensor(out=lv, in0=s12[64:65, :],
                                               scalar=1.0 / DIMK, in1=t2,
                                               op0=AluOpType.mult,
                                               op1=AluOpType.subtract)
                nc.scalar.activation(out=lv, in_=lv, func=AF.Ln, bias=eps1)
                rs = stg.tile([1, 512], BF, tag="rs")
                nc.scalar.activation(out=rs, in_=lv, func=AF.Exp, scale=-0.5)
                # lnriv rows 0/32 = ln(rstd), 1/rstd (engine writes must land
                # on partition 0/32/64); other rows carry stale-but-initialized
                # data that lands in unread transpose columns
                lnriv = lnriv_bufs[t % 2]
                nc.scalar.activation(out=lnriv[32:33, :], in_=lv, func=AF.Exp,
                                     scale=0.5)
                nc.vector.tensor_scalar_mul(lnriv[0:1, :], in0=lv, scalar1=-0.5)
                stats_out[t] = (rs, lnriv)

            def main_stage(t):
                """QKV + RoPE + V transpose for tile t (stats precomputed)."""
                g, half = t // 2, t % 2
                xts = [xg[(k, g)][:, ts(half, 512)] for k in range(KC)]
                rs, lnriv = stats_out.pop(t)
                # broadcast rstd across partitions via a tiny PE outer-product
                rb = qkvp.tile([P, 512], F32, tag="qkv", name=f"rb_{t}")
                nc.tensor.matmul(rb, lhsT=ones1b, rhs=rs,
                                 start=True, stop=True)
                # rstd-scaled rope tables (DVE, early in this unit's queue)
                cs = ts(t % 4, 512)
                ccr = stg.tile([P, 512], BF, tag="ccr")
                nc.vector.tensor_mul(ccr, cc_t[:, cs], rb)
                ssr = stg.tile([P, 512], BF, tag="ssr")
                nc.vector.tensor_mul(ssr, ss_t[:, cs], rb)
                qps = qkvp.tile([P, 512], F32, tag="qkv", name="qps")
                for k in range(KC):
                    nc.tensor.matmul(qps, lhsT=w_t[:, k, ts(0, P)], rhs=xts[k],
                                     start=(k == 0), stop=(k == KC - 1))
                if t < 4:
                    nc.scalar.copy(qn[:, ts(t, 512)], qps)
                else:
                    nc.vector.tensor_copy(qn[:, ts(t, 512)], qps)
                kps = qkvp.tile([P, 512], F32, tag="qkv", name="kps")
                for k in range(KC):
                    nc.tensor.matmul(kps, lhsT=w_t[:, k, ts(1, P)], rhs=xts[k],
                                     start=(k == 0), stop=(k == KC - 1))
                if t < 4:
                    nc.scalar.copy(kn[:, ts(t, 512)], kps)
                else:
                    nc.vector.tensor_copy(kn[:, ts(t, 512)], kps)
                vps = qkvp.tile([P, 512], F32, tag="qkv", name="vps")
                for k in range(KC):
                    nc.tensor.matmul(vps, lhsT=w_t[:, k, ts(2, P)], rhs=xts[k],
                                     start=(k == 0), stop=(k == KC - 1))
                if t < 4:
                    nc.scalar.copy(vn[:, ts(t, 512)], vps)
                else:
                    nc.vector.tensor_copy(vn[:, ts(t, 512)], vps)

                def rope(src):
                    sl = src[:, ts(t, 512)]
                    rp = qkvp.tile([P, 512], F32, tag="qkv")
                    nc.tensor.matmul(rp, lhsT=perm_t, rhs=sl, start=True, stop=True)
                    ra = stg.tile([P, 512], BF, tag="ra")
                    nc.vector.tensor_mul(ra, sl, ccr)
                    rb2 = stg.tile([P, 512], BF, tag="rb2")
                    nc.vector.tensor_mul(rb2, rp, ssr)
                    nc.vector.tensor_add(sl, ra, rb2)

                rope(qn)
                # V "transpose" via regular matmul against identity: output
                # lands in PSUM as f32, which DVE can then 3D-copy to bf16
                # (strided bf16-PSUM reads are rejected by the backend).
                # All matmul groups within one PSUM bank must share the same
                # operand base partition, so head-0 (+lnriv) and head-1
                # blocks go to separate banks.
                tpA = qkvp.tile([P, 8, D], F32, tag="qkv", name="tpA")
                tpB = qkvp.tile([P, 4, D], F32, tag="qkv", name="tpB")
                for j in range(4):
                    nc.tensor.matmul(
                        tpA[:, j, :],
                        lhsT=vn[0:D, 512 * t + P * j : 512 * t + P * (j + 1)],
                        rhs=ident[0:D, 0:D],
                        start=True, stop=True)
                for j in range(4):
                    nc.tensor.matmul(
                        tpA[:, 4 + j, :],
                        lhsT=lnriv[:, P * j : P * (j + 1)],
                        rhs=ident[0:D, 0:D],
                        start=True, stop=True)
                for j in range(4):
                    nc.tensor.matmul(
                        tpB[:, j, :],
                        lhsT=vn[D : 2 * D,
                                512 * t + P * j : 512 * t + P * (j + 1)],
                        rhs=ident[D : 2 * D, D : 2 * D],
                        start=True, stop=True)
                nc.vector.tensor_copy(vb[:, 0, 4 * t : 4 * t + 4, 0:D],
                                      tpA[:, 0:4, :])
                nc.vector.tensor_copy(lnrv[:, 4 * t : 4 * t + 4, 0:1],
                                      tpA[:, 4:8, 0:1])
                nc.vector.tensor_copy(lnrv[:, 4 * t : 4 * t + 4, 1:2],
                                      tpA[:, 4:8, 32:33])
                nc.vector.tensor_copy(vb[:, 1, 4 * t : 4 * t + 4, 0:D],
                                      tpB[:, 0:4, :])
                for h in range(2):
                    nc.gpsimd.tensor_copy(vb[:, h, 4 * t : 4 * t + 4, D : D + 1],
                                          lnrv[:, 4 * t : 4 * t + 4, 1:2])
                rope(kn)

            def sc_group(b, h, J):
                """Scores + exp for key block J of pair (b,h): fills pJ."""
                base = NT * b
                hr = D * h
                m = J % 4
                i0 = 512 * (J // 4)
                ilen = NT - i0
                off0 = P * m
                pJ = pp.tile([P, ilen], BF, tag=f"p{h}_{J}")
                pjs[(h, J)] = pJ
                lhs = kn[hr : hr + D, base + P * J : base + P * (J + 1)]
                for c0 in range(0, ilen, 1024):
                    clen = min(1024, ilen - c0)
                    win = scp.tile([P, 1024], F32, tag="sc")
                    first = c0 == 0
                    lo = off0 if first else 0
                    b0 = lo
                    while b0 < clen:
                        n = min(512 - (b0 % 512), clen - b0)
                        nc.tensor.matmul(
                            win[:, b0 : b0 + n], lhsT=lhs,
                            rhs=qn[hr : hr + D,
                                   base + i0 + c0 + b0 : base + i0 + c0 + b0 + n],
                            start=True, stop=True)
                        b0 += n
                    nc.scalar.activation(out=pJ[:, c0 + lo : c0 + clen],
                                         in_=win[:, lo:clen],
                                         func=AF.Exp, scale=SCALE,
                                         bias=lnrv[:, 16 * b + J, 0:1])
                # causal mask on the diagonal 128x128 block (post-exp)
                nc.gpsimd.tensor_mul(pJ[:, off0 : off0 + P],
                                     pJ[:, off0 : off0 + P], trin_t)

            def av_group(b, h, I):
                """AV for query block I of pair (b,h) + denominator handling."""
                base = NT * b
                hr = D * h
                av = avp.tile([D + 1, 512], F32, tag="av")
                last = 4 * I + 3
                for J in range(0, last + 1):
                    i0 = 512 * (J // 4)
                    cbase = 512 * I - i0
                    off = P * (J % 4) if J // 4 == I else 0
                    nc.tensor.matmul(
                        av[:, off:512],
                        lhsT=vb[:, h, 16 * b + J, :],
                        rhs=pjs[(h, J)][:, cbase + off : cbase + 512],
                        start=(J == 0), stop=(J == last))
                # evict the whole AV tile to SBUF first: frees the single
                # AV PSUM bank ~2us earlier so the next group's matmuls can
                # start while this group's normalize chain drains, and makes
                # the normalize multiply a 2x bf16 op
                av_sb = rrec.tile([D + 1, 512], BF, tag="avsb")
                nc.vector.tensor_copy(av_sb, av)
                rec = rrec.tile([1, 512], BF, tag="rec")
                with nc.allow_low_precision("softmax denom reciprocal in bf16"):
                    nc.vector.reciprocal(rec, av_sb[D : D + 1, :])
                recb = avp.tile([D, 512], F32, tag="recb")
                nc.tensor.matmul(recb, lhsT=ones1b[:, 0:D], rhs=rec,
                                 start=True, stop=True)
                recb_b = rrec.tile([D, 512], BF, tag="recb_b")
                nc.vector.tensor_copy(recb_b, recb)
                nc.vector.tensor_mul(
                    ohT[hr : hr + D, base + 512 * I : base + 512 * (I + 1)],
                    av_sb[0:D, :], recb_b)

            def op_tile(tt_, eng, pool=None):
                """Out-projection for 128-token tile tt_ (0..31)."""
                ps = (pool or opp).tile([P, DIMK], F32, tag="op" if pool is None else "sc")
                for cb in range(2):
                    nc.tensor.matmul(ps[:, ts(cb, 512)],
                                     lhsT=ohT[:, ts(tt_, P)],
                                     rhs=wo_t[:, ts(cb, 512)],
                                     start=True, stop=True)
                ev = evp.tile([P, DIMK], BF, tag="ev")
                if eng == 0:
                    nc.vector.tensor_copy(ev, ps)
                else:
                    nc.vector.tensor_copy(ev[:, 0:512], ps[:, 0:512])
                    nc.scalar.copy(ev[:, 512:1024], ps[:, 512:1024])
                nc.sync.dma_start(out=out_h[ts(tt_, P), :], in_=ev)

            pjs = {}
            sqg = {}

            with tc.tile_pool(name="xtc", bufs=2) as xtc, \
                 tc.tile_pool(name="sqp", bufs=1) as sqp, \
                 tc.tile_pool(name="qkvp", bufs=2, space="PSUM") as qkvp:

                ident = const.tile([P, P], BF)
                nc.gpsimd.memset(ident, 0.0)
                nc.gpsimd.affine_select(out=ident, in_=ident,
                                        compare_op=AluOpType.not_equal, fill=1.0,
                                        base=0, pattern=[[-1, P]],
                                        channel_multiplier=1)

                # startup: w chunk k then x chunk k, so PE starts at chunk 0
                for k in range(KC):
                    nc.sync.dma_start(out=w_t[:, k, :], in_=wq_h[ts(k, P), :])
                    t_ = xtc.tile([P, 1024], BF, tag=f"x{k}")
                    nc.sync.dma_start(out=t_, in_=xt_h[ts(k, P), ts(0, 1024)])
                    xg[(k, 0)] = t_
                nc.sync.dma_start(out=cc_t, in_=cc_h[:, :])
                nc.sync.dma_start(out=ss_t, in_=ss_h[:, :])
                nc.sync.dma_start(out=perm_t, in_=perm_h[:, :])
                xdma(1)
                nc.sync.dma_start(out=trin_t, in_=trin_h[:, :])
                nc.sync.dma_start(out=wo_t, in_=wo_h[:, :])
                stats_stage(0)
                stats_stage(1)
                main_stage(0)
                stats_stage(2)
                xdma(2)
                main_stage(1)
                stats_stage(3)
                main_stage(2)
                stats_stage(4)
                xdma(3)
                main_stage(3)
                stats_stage(5)

                # batch-0 attention interleaved with batch-1 QKV units
                sc_group(0, 0, 0)
                sc_group(0, 1, 0)
                sc_group(0, 0, 1)
                sc_group(0, 1, 1)
                main_stage(4)
                stats_stage(6)
                sc_group(0, 0, 2)
                sc_group(0, 1, 2)
                sc_group(0, 0, 3)
                sc_group(0, 1, 3)
                main_stage(5)
                stats_stage(7)
                sc_group(0, 0, 4)
                sc_group(0, 1, 4)
                sc_group(0, 0, 5)
                sc_group(0, 1, 5)
                main_stage(6)
                sc_group(0, 0, 6)
                sc_group(0, 1, 6)
                sc_group(0, 0, 7)
                sc_group(0, 1, 7)
                main_stage(7)
                av_group(0, 0, 0)
                sc_group(0, 0, 8)
                av_group(0, 1, 0)
                sc_group(0, 1, 8)
                av_group(0, 0, 1)
                sc_group(0, 0, 9)
                av_group(0, 1, 1)
                sc_group(0, 1, 9)
                for J in range(10, 12):
                    sc_group(0, 0, J)
                    sc_group(0, 1, J)
                av_group(0, 0, 2)
                sc_group(0, 0, 12)
                av_group(0, 1, 2)
                sc_group(0, 1, 12)
                for J in range(13, 16):
                    sc_group(0, 0, J)
                    sc_group(0, 1, J)
                av_group(0, 0, 3)
                av_group(0, 1, 3)

            # batch-1 attention with batch-0 out-projection as filler
            with tc.tile_pool(name="opp", bufs=1, space="PSUM") as opp, \
                 tc.tile_pool(name="evp", bufs=2) as evp:
                # out-proj tiles spread one-per-slot between sc groups so the
                # single opp PSUM buffer never stalls PE on its evict
                for J in range(8):
                    sc_group(1, 0, J)
                    op_tile(J, J % 2)
                    sc_group(1, 1, J)
                    op_tile(8 + J, (J + 1) % 2)
                av_group(1, 0, 0)
                sc_group(1, 0, 8)
                av_group(1, 1, 0)
                sc_group(1, 1, 8)
                op_tile(16, 0)
                av_group(1, 0, 1)
                sc_group(1, 0, 9)
                op_tile(17, 1)
                av_group(1, 1, 1)
                sc_group(1, 1, 9)
                op_tile(18, 0)
                sc_group(1, 0, 10)
                op_tile(19, 1)
                sc_group(1, 1, 10)
                sc_group(1, 0, 11)
                sc_group(1, 1, 11)
                av_group(1, 0, 2)
                sc_group(1, 0, 12)
                op_tile(20, 0)
                av_group(1, 1, 2)
                sc_group(1, 1, 12)
                op_tile(21, 1)
                sc_group(1, 0, 13)
                op_tile(22, 0)
                sc_group(1, 1, 13)
                op_tile(23, 1)
                sc_group(1, 0, 14)
                op_tile(24, 0)
                sc_group(1, 1, 14)
                op_tile(25, 1)
                sc_group(1, 0, 15)
                op_tile(26, 0)
                av_group(1, 0, 3)
                sc_group(1, 1, 15)
                op_tile(27, 1)
                av_group(1, 1, 3)
                op_tile(28, 0)
                op_tile(29, 1)
                op_tile(30, 0)
                op_tile(31, 1)

    nc.finalize()
    return nc


def host_inputs(x, W_qkv, W_out, ln_g, ln_b):
    """Per-core input maps (layout/sharding + LN weight folding)."""
    x = np.asarray(x, dtype=np.float32)
    W_qkv = np.asarray(W_qkv, dtype=np.float32)
    W_out = np.asarray(W_out, dtype=np.float32)
    ln_g = np.asarray(ln_g, dtype=np.float32)
    ln_b = np.asarray(ln_b, dtype=np.float32)
    assert np.all(ln_b == 0.0), "bias path not emitted"

    xt = np.ascontiguousarray(x.reshape(T, DIMK).T).astype(BF_NP)  # [1024,4096]

    inv_freq = (1.0 / (10000.0 ** (np.arange(0, D, 2, dtype=np.float32) / D))).astype(np.float32)
    tpos = np.arange(NT, dtype=np.float32)
    freqs = np.outer(tpos, inv_freq).astype(np.float32)     # [2048, 32]
    emb = np.concatenate([freqs, freqs], axis=1)            # [2048, 64]
    cosT = np.cos(emb).T.astype(np.float32)                 # [64, 2048]
    sinT = np.sin(emb).T.astype(np.float32)
    ss_signed = np.concatenate([-sinT[:32], sinT[32:]], axis=0)
    cc = np.ascontiguousarray(np.tile(cosT, (2, 1))).astype(BF_NP)
    ss = np.ascontiguousarray(np.tile(ss_signed, (2, 1))).astype(BF_NP)
    trin = (np.arange(P)[None, :] >= np.arange(P)[:, None]).astype(BF_NP)
    perm = np.zeros((P, P), np.float32)
    for m in range(P):
        blk = (m // D) * D
        perm[blk + (m % D + 32) % D, m] = 1.0
    perm = perm.astype(BF_NP)

    Wg = W_qkv * ln_g[:, None]

    in_maps = []
    for c in range(8):
        cols = np.concatenate([np.arange(P * c, P * (c + 1)),
                               1024 + np.arange(P * c, P * (c + 1)),
                               2048 + np.arange(P * c, P * (c + 1))])
        wl = Wg[:, cols]
        wl = wl - wl.mean(axis=0, keepdims=True)     # fold LN mean-correction
        in_maps.append({
            "xt": xt,
            "wqkv": np.ascontiguousarray(wl).astype(BF_NP),
            "wo": np.ascontiguousarray(W_out[P * c : P * (c + 1), :]).astype(BF_NP),
            "cc": cc, "ss": ss, "trin": trin, "perm": perm,
        })
    return in_maps


_NC_CACHE = {}


def get_program():
    if "nc" not in _NC_CACHE:
        _NC_CACHE["nc"] = build_program()
    return _NC_CACHE["nc"]


LAST_RESULTS = {}


def kernel(x, W_qkv, W_out, b_out, ln_g, ln_b):
    import os
    from concourse.bass_utils import run_bass_kernel_spmd
    nc = get_program()
    in_maps = host_inputs(x, W_qkv, W_out, ln_g, ln_b)
    kw = {}
    if os.environ.get("BASS_KERNEL_TMPDIR"):
        kw["tmpdir"] = os.environ["BASS_KERNEL_TMPDIR"]
    res = run_bass_kernel_spmd(nc, in_maps, list(range(8)), **kw)
    LAST_RESULTS["res"] = res
    total = np.zeros((T, DIMK), dtype=np.float32)
    for r in res.results:
        total += np.asarray(r["out"], dtype=np.float32)
    total += np.asarray(b_out, dtype=np.float32)[None, :]
    return total.reshape(2, NT, DIMK)
